# revision 29
# baseline (speedup 1.0000x reference)
"""Trainium2 Bass kernel for 2-layer GAT (nn_GAT_75273596830284).

Strategy (8 NeuronCores, SPMD, dst-sharded edges):
- Core c owns destination nodes [6250c, 6250c+6250); every edge is processed
  by the core owning its dst, so segment softmax and aggregation are
  exact-local.
- No layer-1 AllGather: each core computes the FULL node table
  h|asrc = x @ [W1 | W1@a_src] for all 50176 (padded) nodes in bf16
  (~0.2 GFLOP/core) and writes it to a private DRAM table. The 53MB
  fp32 AllGather this replaces measured ~4.3ms on this fabric.
- Edge phases gather per-edge rows with indirect DMA (128 rows/instr),
  round-robined over 4 SWDGE queues (BIR post-pass). Per-edge dst-side
  attention values are gathered from small side tables by (static) dst
  index instead of the transpose+matmul one-hot expansion.
- Scatter-accumulate per 128-dst block via one-hot matmul into PSUM in
  bf16 (4x fp32 PE rate), softmax with a constant max-shift
  exp(score-16) (exact: alpha is shift-invariant).
- Layer 2: local table2 [h2|asrc2|adst2] in bf16, 4.2MB AllGather,
  same edge-phase structure.
"""
import sys

sys.path.insert(0, "/opt/trn_rl_repo")
import numpy as np
import orjson

N, E0 = 50000, 800000
NFEAT, HID, NHEAD, NCLASS = 256, 32, 8, 40
NCORES = 8
SH = N // NCORES            # 6250 real nodes per core
P = 128
NB = 49                     # dst blocks per core
SHP = NB * P                # 6272 padded nodes per core
NBG = NCORES * NB           # 392 global blocks
ROWS = NCORES * SHP         # 50176 table rows (remapped node space)
TROW1 = 264                 # table1 row: h(256) + asrc(8)
TROW2 = 42                  # table2 row: h2(40) + asrc2 + adst2
MSHIFT = -16.0              # constant softmax shift (upper bound on scores)
NSWQ = 4                    # SWDGE queues for indirect gathers


# ---------------------------------------------------------------------------
# BIR post-pass: split excess sync waits onto NoOps (walrus rejects >1
# wait/instruction) and round-robin indirect DMAs over the SWDGE queues.
_patched = False


def _install_bir_patch():
    global _patched
    if _patched:
        return
    _patched = True
    from concourse import bass as _bass

    orig = _bass.Bass.to_json_bytes

    def _transform(bir, maxw=1):
        for fn in bir.get("functions", []):
            for bb in fn.get("blocks", []):
                out = []
                qi = 0
                for ins in bb.get("instructions", []):
                    if (ins.get("opcode") == "DMACopy"
                            and ins.get("queue") == "qPoolDynamic"):
                        q = qi % NSWQ
                        qi += 1
                        if q:
                            ins["queue"] = f"qPoolDynamic{q}"
                    si = ins.get("sync_info")
                    ws = (si or {}).get("on_wait") or []
                    if len(ws) > maxw:
                        rest, keep = ws[:-maxw], ws[-maxw:]
                        for i in range(0, len(rest), maxw):
                            out.append({
                                "debug": ins.get("debug", 0),
                                "engine": ins["engine"],
                                "ins": [], "outs": [],
                                "name": f"{ins['name']}-ws{i}",
                                "opcode": "NoOp",
                                "sync_info": {"on_update": [],
                                              "on_wait": rest[i:i + maxw]},
                            })
                        si["on_wait"] = keep
                    out.append(ins)
                bb["instructions"] = out

    def patched(self, *a, **kw):
        bir = orjson.loads(orig(self, *a, **kw))
        _transform(bir)
        return orjson.dumps(bir)

    _bass.Bass.to_json_bytes = patched


# ---------------------------------------------------------------------------
_prog_cache = {}


def _build_program(cpc, phases=5):
    """SPMD Bass program. phases: 1=N1, 2=+E1, 3=+N2, 4=+AG2, 5=+E2+out."""
    import concourse.bass as bass
    import concourse.tile as tile
    from concourse import mybir
    from contextlib import ExitStack

    f32 = mybir.dt.float32
    bf16 = mybir.dt.bfloat16
    i32 = mybir.dt.int32
    TOT = NB * cpc

    nc = bass.Bass("TRN2", target_bir_lowering=False, debug=False,
                   num_devices=NCORES, num_swdge_queues=NSWQ)

    def din(name, shape, dt=f32):
        return nc.dram_tensor(name, shape, dt, kind="ExternalInput").ap()

    xT = din("xT", [NFEAT, ROWS], bf16)           # replicated, remapped order
    wcat1 = din("wcat1", [NFEAT, 272], bf16)      # [W1 | W1@As | W1@Ad]
    w2cat = din("w2cat", [NFEAT, TROW2], bf16)    # [W2 | W2@as2 | W2@ad2]
    b1rep = din("b1rep", [P, 256])
    b2rep = din("b2rep", [P, NCLASS])
    iota_in = din("iota_row", [P, P], bf16)
    ident_in = din("ident", [P, P], bf16)
    ld_in = din("ld", [P, TOT], bf16)
    six_in = din("srcidx", [P, TOT], i32)         # remapped global src
    dix_in = din("dstidxg", [P, TOT], i32)        # remapped global dst
    dixl_in = din("dstidxl", [P, TOT], i32)       # local dst (0..6271)
    adbi_in = din("adblkidx", [P, NB], i32)       # row c*SHP+b*128+p

    tbl1 = nc.dram_tensor("tbl1", [ROWS, TROW1], bf16).ap()
    ad1t = nc.dram_tensor("ad1t", [ROWS, 8], bf16).ap()
    t2shard = nc.dram_tensor("t2shard", [SHP, TROW2], bf16).ap()
    t2full = nc.dram_tensor("t2full", [ROWS, TROW2], bf16,
                            addr_space="Shared").ap()

    fin = nc.dram_tensor("fin", [SHP, NCLASS], f32, kind="ExternalOutput").ap()
    lsm = nc.dram_tensor("lsm", [SHP, NCLASS], f32, kind="ExternalOutput").ap()

    AL = mybir.AluOpType
    AF = mybir.ActivationFunctionType
    groups = [list(range(NCORES))]

    with tile.TileContext(nc, num_cores=NCORES) as tc, ExitStack() as ctx:
        perm = ctx.enter_context(tc.tile_pool(name="perm", bufs=1))
        iota_sb = perm.tile([P, P], bf16)
        ident_sb = perm.tile([P, P], bf16)
        b1_sb = perm.tile([P, 256], f32)
        b2_sb = perm.tile([P, NCLASS], f32)
        ld_sb = perm.tile([P, TOT], bf16)
        six_sb = perm.tile([P, TOT], i32)
        dix_sb = perm.tile([P, TOT], i32)
        dixl_sb = perm.tile([P, TOT], i32)
        adbi_sb = perm.tile([P, NB], i32)
        w1_sb = perm.tile([P, 2, 272], bf16)
        w2_sb = perm.tile([P, 2, TROW2], bf16)
        h1_sb = perm.tile([P, NB * 256], bf16)
        h1t0 = perm.tile([P, SHP], bf16)
        h1t1 = perm.tile([P, SHP], bf16)
        o2_sb = perm.tile([P, NB * NCLASS], f32)
        mmax_sb = perm.tile([P, NB], f32)
        ssum_sb = perm.tile([P, NB], f32)
        raw1_sb = perm.tile([P, NB * 256], bf16)
        rcp1_sb = perm.tile([P, NB * 8], f32)
        raw2_sb = perm.tile([P, NB * NCLASS], bf16)
        rcp2_sb = perm.tile([P, NB], f32)
        msh_sb = perm.tile([P, 1], f32)
        nc.gpsimd.memset(msh_sb[:], MSHIFT)

        nc.sync.dma_start(out=iota_sb[:], in_=iota_in[:])
        nc.sync.dma_start(out=ident_sb[:], in_=ident_in[:])
        nc.sync.dma_start(out=b1_sb[:], in_=b1rep[:])
        nc.sync.dma_start(out=b2_sb[:], in_=b2rep[:])
        nc.sync.dma_start(out=ld_sb[:], in_=ld_in[:])
        nc.sync.dma_start(out=six_sb[:], in_=six_in[:])
        nc.sync.dma_start(out=dix_sb[:], in_=dix_in[:])
        nc.sync.dma_start(out=dixl_sb[:], in_=dixl_in[:])
        nc.sync.dma_start(out=adbi_sb[:], in_=adbi_in[:])
        nc.sync.dma_start(out=w1_sb[:, 0, :], in_=wcat1[0:P, :])
        nc.sync.dma_start(out=w1_sb[:, 1, :], in_=wcat1[P:2 * P, :])
        nc.sync.dma_start(out=w2_sb[:, 0, :], in_=w2cat[0:P, :])
        nc.sync.dma_start(out=w2_sb[:, 1, :], in_=w2cat[P:2 * P, :])

        # ---------------- N1: full node table (all 392 blocks, each core)
        GB = 14                                 # blocks per x-stream group
        with tc.tile_pool(name="xt", bufs=3) as xtp, \
             tc.tile_pool(name="hx", bufs=8) as hxp, \
             tc.tile_pool(name="pn1", bufs=6, space="PSUM") as pn1:
            for g in range(NBG // GB):
                xg = xtp.tile([P, 2, GB * P], bf16, tag="xg")
                c0 = g * GB * P
                nc.sync.dma_start(out=xg[:, 0, :], in_=xT[0:P, c0:c0 + GB * P])
                nc.scalar.dma_start(out=xg[:, 1, :],
                                 in_=xT[P:2 * P, c0:c0 + GB * P])
                for nb in range(GB):
                    gb = g * GB + nb
                    ph = pn1.tile([P, 272], f32, tag="ph")
                    nc.tensor.matmul(out=ph[:], lhsT=xg[:, 0, nb * P:(nb + 1) * P],
                                     rhs=w1_sb[:, 0, :], start=True, stop=False)
                    nc.tensor.matmul(out=ph[:], lhsT=xg[:, 1, nb * P:(nb + 1) * P],
                                     rhs=w1_sb[:, 1, :], start=False, stop=True)
                    hx = hxp.tile([P, 272], bf16, tag="hx")
                    if gb % 2 == 0:
                        nc.scalar.activation(hx[:], ph[:], AF.Copy)
                    else:
                        nc.vector.tensor_copy(hx[:], ph[:])
                    nc.sync.dma_start(out=tbl1[gb * P:(gb + 1) * P, :],
                                      in_=hx[:, 0:TROW1])
                    nc.scalar.dma_start(out=ad1t[gb * P:(gb + 1) * P, :],
                                     in_=hx[:, TROW1:272])

        # ---------------- E1: layer-1 edge phase
        with tc.tile_pool(name="gp", bufs=4) as gp, \
             tc.tile_pool(name="ohp", bufs=4) as ohp, \
             tc.tile_pool(name="sp", bufs=10) as sp, \
             tc.tile_pool(name="fp", bufs=3) as fp, \
             tc.tile_pool(name="oht", bufs=2) as ohtp, \
             tc.tile_pool(name="adb", bufs=4) as adbp, \
             tc.tile_pool(name="pb", bufs=3, space="PSUM") as pbp, \
             tc.tile_pool(name="pt", bufs=1, space="PSUM") as ptp, \
             tc.tile_pool(name="pa", bufs=1, space="PSUM") as pap, \
             tc.tile_pool(name="psb", bufs=1, space="PSUM") as psbp:
            for b in range(NB if phases >= 2 else 0):
                pblk = pbp.tile([P, TROW1], f32, tag="pblk")
                g1 = gp.tile([P, cpc, TROW1], bf16, tag="g")
                adb = adbp.tile([P, 8], bf16, tag="adb")
                nc.gpsimd.indirect_dma_start(
                    out=adb[:], out_offset=None, in_=ad1t[:],
                    in_offset=bass.IndirectOffsetOnAxis(
                        ap=adbi_sb[:, b:b + 1], axis=0))
                for ch in range(cpc):
                    t = b * cpc + ch
                    nc.gpsimd.indirect_dma_start(
                        out=g1[:, ch, :], out_offset=None, in_=tbl1[:],
                        in_offset=bass.IndirectOffsetOnAxis(
                            ap=six_sb[:, t:t + 1], axis=0))
                oh = ohp.tile([P, cpc, P], bf16, tag="oh")
                nc.vector.tensor_tensor(
                    out=oh[:],
                    in0=ld_sb[:, b * cpc:(b + 1) * cpc].unsqueeze(-1)
                        .to_broadcast([P, cpc, P]),
                    in1=iota_sb[:].unsqueeze(1).to_broadcast([P, cpc, P]),
                    op=AL.is_equal)
                pa_blk = pap.tile([P, cpc * 8], f32, tag="pa")
                pstb = psbp.tile([P, cpc, P], bf16, tag="pstb")
                for ch in range(cpc):
                    nc.tensor.transpose(out=pstb[:, ch, :], in_=oh[:, ch, :],
                                        identity=ident_sb[:])
                ohtb = ohtp.tile([P, cpc, P], bf16, tag="ohtb")
                nc.scalar.activation(ohtb[:], pstb[:], AF.Copy)
                for ch in range(cpc):
                    nc.tensor.matmul(out=pa_blk[:, ch * 8:(ch + 1) * 8],
                                     lhsT=ohtb[:, ch, :], rhs=adb[:],
                                     start=True, stop=True)
                hsp = (cpc + 1) // 2
                for lo, hi in ((0, hsp), (hsp, cpc)):
                    w = hi - lo
                    pa_v = pa_blk[:, lo * 8:hi * 8]
                    esc = sp.tile([P, w, 8], f32, tag="esc")
                    nc.vector.tensor_tensor(
                        out=esc[:], in0=g1[:, lo:hi, 256:264],
                        in1=pa_v.rearrange("p (t h) -> p t h", t=w),
                        op=AL.add)
                    es2 = sp.tile([P, w, 8], f32, tag="es2")
                    nc.vector.tensor_scalar_mul(es2[:], esc[:], 0.2)
                    es3 = sp.tile([P, w, 8], f32, tag="es3")
                    nc.vector.tensor_tensor(out=es3[:], in0=esc[:], in1=es2[:],
                                            op=AL.max)
                    nc.scalar.activation(g1[:, lo:hi, 256:264], es3[:], AF.Exp,
                                         bias=msh_sb[:])
                    nc.vector.tensor_tensor(
                        out=g1[:, lo:hi, 0:256]
                            .rearrange("p t (h c) -> p t h c", h=8),
                        in0=g1[:, lo:hi, 0:256]
                            .rearrange("p t (h c) -> p t h c", h=8),
                        in1=g1[:, lo:hi, 256:264].unsqueeze(-1)
                            .to_broadcast([P, w, 8, 32]),
                        op=AL.mult)
                for ch in range(cpc):
                    nc.tensor.matmul(out=pblk[:], lhsT=oh[:, ch, :],
                                     rhs=g1[:, ch, :],
                                     start=(ch == 0), stop=(ch == cpc - 1))
                # park accumulator: reciprocal of denominators + raw sums
                den = sp.tile([P, 8], f32, tag="den")
                nc.vector.tensor_scalar_add(den[:], pblk[:, 256:264], 1e-16)
                nc.vector.reciprocal(rcp1_sb[:, b * 8:(b + 1) * 8], den[:])
                nc.scalar.activation(raw1_sb[:, b * 256:(b + 1) * 256],
                                     pblk[:, 0:256], AF.Copy)

            # pass B: alpha normalize + bias + elu -> h1 (1-block skew so the
            # DVE never waits on the scalar exp of the same block)
            def pb_front(b):
                xb = fp.tile([P, 256], f32, tag="xb")
                nc.vector.tensor_tensor(
                    out=xb[:].rearrange("p (h c) -> p h c", h=8),
                    in0=raw1_sb[:, b * 256:(b + 1) * 256]
                        .rearrange("p (h c) -> p h c", h=8),
                    in1=rcp1_sb[:, b * 8:(b + 1) * 8].unsqueeze(-1)
                        .to_broadcast([P, 8, 32]),
                    op=AL.mult)
                nc.vector.tensor_tensor(out=xb[:], in0=xb[:], in1=b1_sb[:],
                                        op=AL.add)
                m0 = fp.tile([P, 256], f32, tag="m0")
                nc.vector.tensor_scalar(out=m0[:], in0=xb[:], scalar1=0.0,
                                        scalar2=None, op0=AL.min)
                ex = fp.tile([P, 256], f32, tag="ex")
                nc.scalar.activation(ex[:], m0[:], AF.Exp)
                nc.vector.tensor_scalar(out=m0[:], in0=xb[:], scalar1=0.0,
                                        scalar2=None, op0=AL.max)
                return b, m0, ex

            def pb_back(st):
                b, m0, ex = st
                nc.vector.tensor_scalar_add(ex[:], ex[:], -1.0)
                h1b = h1_sb[:, b * 256:(b + 1) * 256]
                nc.vector.tensor_tensor(out=h1b, in0=m0[:], in1=ex[:],
                                        op=AL.add)

            prevb = None
            for b in range(NB if phases >= 2 else 0):
                stb = pb_front(b)
                if prevb is not None:
                    pb_back(prevb)
                prevb = stb
            if prevb is not None:
                pb_back(prevb)

            # h1 transposes fused with the layer-2 node-table matmuls
            for b in range(NB if phases >= 2 else 0):
                for half, dstt in ((0, h1t0), (1, h1t1)):
                    pst2 = ptp.tile([P, P], bf16, tag="pst")
                    nc.tensor.transpose(
                        out=pst2[:],
                        in_=h1_sb[:, b * 256 + half * P: b * 256 + (half + 1) * P],
                        identity=ident_sb[:])
                    nc.scalar.activation(dstt[:, b * P:(b + 1) * P], pst2[:],
                                         AF.Copy)
                if phases >= 3:
                    p2 = ptp.tile([P, TROW2], f32, tag="pst")
                    nc.tensor.matmul(out=p2[:], lhsT=h1t0[:, b * P:(b + 1) * P],
                                     rhs=w2_sb[:, 0, :], start=True, stop=False)
                    nc.tensor.matmul(out=p2[:], lhsT=h1t1[:, b * P:(b + 1) * P],
                                     rhs=w2_sb[:, 1, :], start=False, stop=True)
                    h2x = sp.tile([P, TROW2], bf16, tag="h2x")
                    nc.scalar.activation(h2x[:], p2[:], AF.Copy)
                    nc.sync.dma_start(out=t2shard[b * P:(b + 1) * P, :],
                                      in_=h2x[:])

            # ---------------- AG2
            if phases >= 4:
                nc.gpsimd.collective_compute(
                    "AllGather", AL.bypass, replica_groups=groups,
                    ins=[t2shard[:]], outs=[t2full[:]])

            # ---------------- E2: layer-2 edge phase
            for b in range(NB if phases >= 5 else 0):
                pblk2 = pbp.tile([P, 41], f32, tag="pblk")
                g2 = gp.tile([P, cpc, TROW2], bf16, tag="g2")
                adb2 = adbp.tile([P, TROW2], bf16, tag="adb2")
                nc.sync.dma_start(out=adb2[:],
                                  in_=t2shard[b * P:(b + 1) * P, :])
                for ch in range(cpc):
                    t = b * cpc + ch
                    nc.gpsimd.indirect_dma_start(
                        out=g2[:, ch, :], out_offset=None, in_=t2full[:],
                        in_offset=bass.IndirectOffsetOnAxis(
                            ap=six_sb[:, t:t + 1], axis=0))
                oh = ohp.tile([P, cpc, P], bf16, tag="oh")
                nc.vector.tensor_tensor(
                    out=oh[:],
                    in0=ld_sb[:, b * cpc:(b + 1) * cpc].unsqueeze(-1)
                        .to_broadcast([P, cpc, P]),
                    in1=iota_sb[:].unsqueeze(1).to_broadcast([P, cpc, P]),
                    op=AL.is_equal)
                pa2 = pap.tile([P, cpc], f32, tag="pa")
                pstb = psbp.tile([P, cpc, P], bf16, tag="pstb")
                for ch in range(cpc):
                    nc.tensor.transpose(out=pstb[:, ch, :], in_=oh[:, ch, :],
                                        identity=ident_sb[:])
                ohtb = ohtp.tile([P, cpc, P], bf16, tag="ohtb")
                nc.scalar.activation(ohtb[:], pstb[:], AF.Copy)
                for ch in range(cpc):
                    nc.tensor.matmul(out=pa2[:, ch:ch + 1],
                                     lhsT=ohtb[:, ch, :], rhs=adb2[:, 41:42],
                                     start=True, stop=True)
                hsp = (cpc + 1) // 2
                for lo, hi in ((0, hsp), (hsp, cpc)):
                    w = hi - lo
                    esc = sp.tile([P, w], f32, tag="esc1")
                    nc.vector.tensor_tensor(
                        out=esc[:],
                        in0=g2[:, lo:hi, 40:41].rearrange("p t o -> p (t o)"),
                        in1=pa2[:, lo:hi], op=AL.add)
                    es2 = sp.tile([P, w], f32, tag="es21")
                    nc.vector.tensor_scalar_mul(es2[:], esc[:], 0.2)
                    es3 = sp.tile([P, w], f32, tag="es31")
                    nc.vector.tensor_tensor(out=es3[:], in0=esc[:], in1=es2[:],
                                            op=AL.max)
                    nc.scalar.activation(
                        g2[:, lo:hi, 40:41].rearrange("p t o -> p (t o)"),
                        es3[:], AF.Exp, bias=msh_sb[:])
                    nc.vector.tensor_tensor(
                        out=g2[:, lo:hi, 0:40], in0=g2[:, lo:hi, 0:40],
                        in1=g2[:, lo:hi, 40:41].to_broadcast([P, w, 40]),
                        op=AL.mult)
                for ch in range(cpc):
                    nc.tensor.matmul(out=pblk2[:], lhsT=oh[:, ch, :],
                                     rhs=g2[:, ch, 0:41],
                                     start=(ch == 0), stop=(ch == cpc - 1))
                den = sp.tile([P, 1], f32, tag="den1")
                nc.vector.tensor_scalar_add(den[:], pblk2[:, 40:41], 1e-16)
                nc.vector.reciprocal(rcp2_sb[:, b:b + 1], den[:])
                nc.scalar.activation(raw2_sb[:, b * NCLASS:(b + 1) * NCLASS],
                                     pblk2[:, 0:40], AF.Copy)

            # E2 pass B: normalize + bias + row-max + exp-sum
            for b in range(NB if phases >= 5 else 0):
                o2 = o2_sb[:, b * NCLASS:(b + 1) * NCLASS]
                nc.vector.tensor_tensor(
                    out=o2, in0=raw2_sb[:, b * NCLASS:(b + 1) * NCLASS],
                    in1=rcp2_sb[:, b:b + 1].to_broadcast([P, 40]),
                    op=AL.mult)
                nc.vector.tensor_tensor(out=o2, in0=o2, in1=b2_sb[:], op=AL.add)
                nc.vector.tensor_reduce(out=mmax_sb[:, b:b + 1], in_=o2,
                                        axis=mybir.AxisListType.X, op=AL.max)
                negm = sp.tile([P, 1], f32, tag="negm")
                nc.vector.tensor_scalar_mul(negm[:], mmax_sb[:, b:b + 1], -1.0)
                et = sp.tile([P, NCLASS], f32, tag="et")
                nc.scalar.activation(et[:], o2, AF.Exp, bias=negm[:],
                                     accum_out=ssum_sb[:, b:b + 1])

            if phases >= 5:
                lnS = perm.tile([P, NB], f32)
                nc.scalar.activation(lnS[:], ssum_sb[:], AF.Ln)
                q = perm.tile([P, NB], f32)
                nc.vector.tensor_tensor(out=q[:], in0=mmax_sb[:], in1=lnS[:],
                                        op=AL.add)
                for b in range(NB):
                    o2 = o2_sb[:, b * NCLASS:(b + 1) * NCLASS]
                    lsb = sp.tile([P, NCLASS], f32, tag="lsb")
                    nc.vector.tensor_tensor(out=lsb[:], in0=o2,
                                            in1=q[:, b:b + 1].to_broadcast([P, 40]),
                                            op=AL.subtract)
                    nc.sync.dma_start(out=fin[b * P:(b + 1) * P, :], in_=o2)
                    nc.scalar.dma_start(out=lsm[b * P:(b + 1) * P, :], in_=lsb[:])
    return nc


# ---------------------------------------------------------------------------
def _host_prep(x, edge_index, W1, att_src1, att_dst1, b1, W2, att_src2,
               att_dst2, b2):
    import ml_dtypes
    bf = ml_dtypes.bfloat16

    x = np.asarray(x, dtype=np.float32)
    ei = np.asarray(edge_index)
    W1 = np.asarray(W1, np.float32)
    W2 = np.asarray(W2, np.float32)
    a_s1 = np.asarray(att_src1, np.float32)
    a_d1 = np.asarray(att_dst1, np.float32)
    a_s2 = np.asarray(att_src2, np.float32).reshape(-1)
    a_d2 = np.asarray(att_dst2, np.float32).reshape(-1)
    b1 = np.asarray(b1, np.float32)
    b2 = np.asarray(b2, np.float32)

    loops = np.arange(N, dtype=ei.dtype)
    src = np.concatenate([ei[0], loops])
    dst = np.concatenate([ei[1], loops])
    order = np.argsort(dst, kind="stable")
    src_s = src[order].astype(np.int64)
    dst_s = dst[order].astype(np.int64)

    core = dst_s // SH
    local = dst_s - core * SH
    block = local // P
    slot = local % P
    cb = core * NB + block
    counts = np.bincount(cb, minlength=NCORES * NB)
    cpc = int(np.ceil(counts.max() / P))
    TOT = NB * cpc

    offs = np.zeros(NCORES * NB + 1, np.int64)
    offs[1:] = np.cumsum(counts)
    pos = np.arange(src_s.size, dtype=np.int64) - offs[cb]
    chunk = pos // P
    lane = pos % P
    col = block * cpc + chunk

    ld_all = np.full((NCORES, P, TOT), -1.0, np.float32)
    six_all = np.zeros((NCORES, P, TOT), np.int32)
    dix_all = np.zeros((NCORES, P, TOT), np.int32)
    dixl_all = np.zeros((NCORES, P, TOT), np.int32)
    smap = ((src_s // SH) * SHP + (src_s % SH)).astype(np.int32)
    dmap = (core * SHP + local).astype(np.int32)
    ld_all[core, lane, col] = slot.astype(np.float32)
    six_all[core, lane, col] = smap
    dix_all[core, lane, col] = dmap
    dixl_all[core, lane, col] = (local).astype(np.int32)

    # weights folded with attention vectors
    wa_s1 = np.zeros((NFEAT, 8), np.float32)
    wa_d1 = np.zeros((NFEAT, 8), np.float32)
    for h in range(NHEAD):
        wa_s1[:, h] = W1[:, h * HID:(h + 1) * HID] @ a_s1[h]
        wa_d1[:, h] = W1[:, h * HID:(h + 1) * HID] @ a_d1[h]
    wcat1 = np.concatenate([W1, wa_s1, wa_d1], axis=1).astype(bf)   # [256,272]
    w2cat = np.concatenate([W2, (W2 @ a_s2)[:, None],
                            (W2 @ a_d2)[:, None]], axis=1).astype(bf)

    # replicated x in remapped (core-padded) row order, transposed
    xs_all = np.zeros((ROWS, NFEAT), np.float32)
    for c in range(NCORES):
        xs_all[c * SHP:c * SHP + SH] = x[c * SH:(c + 1) * SH]
    xT = np.ascontiguousarray(xs_all.T).astype(bf)

    iota_row = np.broadcast_to(np.arange(P, dtype=np.float32),
                               (P, P)).astype(bf)
    ident = np.eye(P, dtype=np.float32).astype(bf)
    b1rep = np.broadcast_to(b1, (P, 256)).copy()
    b2rep = np.broadcast_to(b2, (P, NCLASS)).copy()

    pp = np.arange(P, dtype=np.int32)
    in_maps = []
    for c in range(NCORES):
        adblkidx = (c * SHP + (np.arange(NB, dtype=np.int32) * P)[None, :]
                    + pp[:, None])
        in_maps.append({
            "adblkidx": np.ascontiguousarray(adblkidx),
            "xT": xT,
            "wcat1": wcat1, "w2cat": w2cat,
            "b1rep": b1rep, "b2rep": b2rep,
            "iota_row": np.ascontiguousarray(iota_row), "ident": ident,
            "ld": np.ascontiguousarray(ld_all[c]).astype(bf),
            "srcidx": np.ascontiguousarray(six_all[c]),
            "dstidxg": np.ascontiguousarray(dix_all[c]),
            "dstidxl": np.ascontiguousarray(dixl_all[c]),
        })
    return cpc, in_maps


def kernel(**inputs):
    _install_bir_patch()
    from concourse.bass_utils import run_bass_kernel_spmd

    cpc, in_maps = _host_prep(
        inputs["x"], inputs["edge_index"], inputs["W1"], inputs["att_src1"],
        inputs["att_dst1"], inputs["b1"], inputs["W2"], inputs["att_src2"],
        inputs["att_dst2"], inputs["b2"])

    if cpc not in _prog_cache:
        _prog_cache[cpc] = _build_program(cpc)
    nc = _prog_cache[cpc]

    res = run_bass_kernel_spmd(nc, in_maps, list(range(NCORES)))
    fin = np.concatenate([res.results[c]["fin"][:SH] for c in range(NCORES)])
    lsm = np.concatenate([res.results[c]["lsm"][:SH] for c in range(NCORES)])
    return fin, lsm


# revision 30
# speedup vs baseline: 1.3298x; 1.3298x over previous
"""Trainium2 Bass kernel for 2-layer GAT (nn_GAT_75273596830284).

Strategy (8 NeuronCores, SPMD, dst-sharded edges):
- Core c owns destination nodes [6250c, 6250c+6250); every edge is processed
  by the core owning its dst, so segment softmax and aggregation are
  exact-local.
- No layer-1 AllGather: each core computes the FULL node table
  h|asrc = x @ [W1 | W1@a_src] for all 50176 (padded) nodes in bf16
  (~0.2 GFLOP/core) and writes it to a private DRAM table. The 53MB
  fp32 AllGather this replaces measured ~4.3ms on this fabric.
- Edge phases gather per-edge rows with indirect DMA (128 rows/instr),
  round-robined over 4 SWDGE queues (BIR post-pass). Per-edge dst-side
  attention values are gathered from small side tables by (static) dst
  index instead of the transpose+matmul one-hot expansion.
- Scatter-accumulate per 128-dst block via one-hot matmul into PSUM in
  bf16 (4x fp32 PE rate), softmax with a constant max-shift
  exp(score-16) (exact: alpha is shift-invariant).
- Layer 2: local table2 [h2|asrc2|adst2] in bf16, 4.2MB AllGather,
  same edge-phase structure.
"""
import sys

sys.path.insert(0, "/opt/trn_rl_repo")
import numpy as np
import orjson

N, E0 = 50000, 800000
NFEAT, HID, NHEAD, NCLASS = 256, 32, 8, 40
NCORES = 8
SH = N // NCORES            # 6250 real nodes per core
P = 128
NB = 49                     # dst blocks per core
SHP = NB * P                # 6272 padded nodes per core
NBG = NCORES * NB           # 392 global blocks
ROWS = NCORES * SHP         # 50176 table rows (remapped node space)
TROW1 = 264                 # table1 row: h(256) + asrc(8)
TROW2 = 42                  # table2 row: h2(40) + asrc2 + adst2
MSHIFT = -16.0              # constant softmax shift (upper bound on scores)
NSWQ = 4                    # SWDGE queues for indirect gathers


# ---------------------------------------------------------------------------
# BIR post-pass: split excess sync waits onto NoOps (walrus rejects >1
# wait/instruction) and round-robin indirect DMAs over the SWDGE queues.
_patched = False


def _install_bir_patch():
    global _patched
    if _patched:
        return
    _patched = True
    from concourse import bass as _bass

    orig = _bass.Bass.to_json_bytes

    def _transform(bir, maxw=1):
        for fn in bir.get("functions", []):
            for bb in fn.get("blocks", []):
                out = []
                qi = 0
                for ins in bb.get("instructions", []):
                    if (ins.get("opcode") == "DMACopy"
                            and ins.get("queue") == "qPoolDynamic"):
                        q = qi % NSWQ
                        qi += 1
                        if q:
                            ins["queue"] = f"qPoolDynamic{q}"
                    si = ins.get("sync_info")
                    ws = (si or {}).get("on_wait") or []
                    if len(ws) > maxw:
                        rest, keep = ws[:-maxw], ws[-maxw:]
                        for i in range(0, len(rest), maxw):
                            out.append({
                                "debug": ins.get("debug", 0),
                                "engine": ins["engine"],
                                "ins": [], "outs": [],
                                "name": f"{ins['name']}-ws{i}",
                                "opcode": "NoOp",
                                "sync_info": {"on_update": [],
                                              "on_wait": rest[i:i + maxw]},
                            })
                        si["on_wait"] = keep
                    out.append(ins)
                bb["instructions"] = out

    def patched(self, *a, **kw):
        bir = orjson.loads(orig(self, *a, **kw))
        _transform(bir)
        return orjson.dumps(bir)

    _bass.Bass.to_json_bytes = patched


# ---------------------------------------------------------------------------
_prog_cache = {}


def _build_program(cpc, phases=5):
    """SPMD Bass program. phases: 1=N1, 2=+E1, 3=+N2, 4=+AG2, 5=+E2+out."""
    import concourse.bass as bass
    import concourse.tile as tile
    from concourse import mybir
    from contextlib import ExitStack

    f32 = mybir.dt.float32
    bf16 = mybir.dt.bfloat16
    i32 = mybir.dt.int32
    TOT = NB * cpc

    nc = bass.Bass("TRN2", target_bir_lowering=False, debug=False,
                   num_devices=NCORES, num_swdge_queues=NSWQ)

    def din(name, shape, dt=f32):
        return nc.dram_tensor(name, shape, dt, kind="ExternalInput").ap()

    xT = din("xT", [NFEAT, ROWS], bf16)           # replicated, remapped order
    wcat1 = din("wcat1", [NFEAT, 272], bf16)      # [W1 | W1@As | W1@Ad]
    w2cat = din("w2cat", [NFEAT, TROW2], bf16)    # [W2 | W2@as2 | W2@ad2]
    b1rep = din("b1rep", [P, 256])
    b2rep = din("b2rep", [P, NCLASS])
    iota_in = din("iota_row", [P, P], bf16)
    ident_in = din("ident", [P, P], bf16)
    ld_in = din("ld", [P, TOT], bf16)
    six_in = din("srcidx", [P, TOT], i32)         # remapped global src
    dix_in = din("dstidxg", [P, TOT], i32)        # remapped global dst
    dixl_in = din("dstidxl", [P, TOT], i32)       # local dst (0..6271)
    adbi_in = din("adblkidx", [P, NB], i32)       # row c*SHP+b*128+p

    tbl1 = nc.dram_tensor("tbl1", [ROWS, TROW1], bf16).ap()
    ad1t = nc.dram_tensor("ad1t", [ROWS, 8], bf16).ap()
    t2shard = nc.dram_tensor("t2shard", [SHP, TROW2], bf16).ap()
    t2full = nc.dram_tensor("t2full", [ROWS, TROW2], bf16,
                            addr_space="Shared").ap()

    fin = nc.dram_tensor("fin", [SHP, NCLASS], f32, kind="ExternalOutput").ap()
    lsm = nc.dram_tensor("lsm", [SHP, NCLASS], f32, kind="ExternalOutput").ap()

    AL = mybir.AluOpType
    AF = mybir.ActivationFunctionType
    groups = [list(range(NCORES))]

    with tile.TileContext(nc, num_cores=NCORES) as tc, ExitStack() as ctx:
        perm = ctx.enter_context(tc.tile_pool(name="perm", bufs=1))
        iota_sb = perm.tile([P, P], bf16)
        ident_sb = perm.tile([P, P], bf16)
        b1_sb = perm.tile([P, 256], f32)
        b2_sb = perm.tile([P, NCLASS], f32)
        ld_sb = perm.tile([P, TOT], bf16)
        six_sb = perm.tile([P, TOT], i32)
        dix_sb = perm.tile([P, TOT], i32)
        dixl_sb = perm.tile([P, TOT], i32)
        adbi_sb = perm.tile([P, NB], i32)
        w1_sb = perm.tile([P, 2, 272], bf16)
        w2_sb = perm.tile([P, 2, TROW2], bf16)
        h1_sb = perm.tile([P, NB * 256], bf16)
        h1t0 = perm.tile([P, SHP], bf16)
        h1t1 = perm.tile([P, SHP], bf16)
        o2_sb = perm.tile([P, NB * NCLASS], f32)
        mmax_sb = perm.tile([P, NB], f32)
        ssum_sb = perm.tile([P, NB], f32)
        raw1_sb = perm.tile([P, NB * 256], bf16)
        rcp1_sb = perm.tile([P, NB * 8], f32)
        raw2_sb = perm.tile([P, NB * NCLASS], bf16)
        rcp2_sb = perm.tile([P, NB], f32)
        msh_sb = perm.tile([P, 1], f32)
        nc.gpsimd.memset(msh_sb[:], MSHIFT)

        nc.sync.dma_start(out=iota_sb[:], in_=iota_in[:])
        nc.sync.dma_start(out=ident_sb[:], in_=ident_in[:])
        nc.sync.dma_start(out=b1_sb[:], in_=b1rep[:])
        nc.sync.dma_start(out=b2_sb[:], in_=b2rep[:])
        nc.sync.dma_start(out=ld_sb[:], in_=ld_in[:])
        nc.sync.dma_start(out=six_sb[:], in_=six_in[:])
        nc.sync.dma_start(out=dix_sb[:], in_=dix_in[:])
        nc.sync.dma_start(out=dixl_sb[:], in_=dixl_in[:])
        nc.sync.dma_start(out=adbi_sb[:], in_=adbi_in[:])
        nc.sync.dma_start(out=w1_sb[:, 0, :], in_=wcat1[0:P, :])
        nc.sync.dma_start(out=w1_sb[:, 1, :], in_=wcat1[P:2 * P, :])
        nc.sync.dma_start(out=w2_sb[:, 0, :], in_=w2cat[0:P, :])
        nc.sync.dma_start(out=w2_sb[:, 1, :], in_=w2cat[P:2 * P, :])

        # ---------------- N1: full node table (all 392 blocks, each core)
        GB = 8                                   # blocks per x-stream group
        with tc.tile_pool(name="xt", bufs=3) as xtp, \
             tc.tile_pool(name="hx", bufs=8) as hxp, \
             tc.tile_pool(name="pn1", bufs=6, space="PSUM") as pn1:
            for g in range(NBG // GB):
                xg = xtp.tile([P, 2, GB * P], bf16, tag="xg")
                c0 = g * GB * P
                nc.sync.dma_start(out=xg[:, 0, :], in_=xT[0:P, c0:c0 + GB * P])
                nc.scalar.dma_start(out=xg[:, 1, :],
                                 in_=xT[P:2 * P, c0:c0 + GB * P])
                for nb in range(GB):
                    gb = g * GB + nb
                    ph = pn1.tile([P, 272], f32, tag="ph")
                    nc.tensor.matmul(out=ph[:], lhsT=xg[:, 0, nb * P:(nb + 1) * P],
                                     rhs=w1_sb[:, 0, :], start=True, stop=False)
                    nc.tensor.matmul(out=ph[:], lhsT=xg[:, 1, nb * P:(nb + 1) * P],
                                     rhs=w1_sb[:, 1, :], start=False, stop=True)
                    hx = hxp.tile([P, 272], bf16, tag="hx")
                    if gb % 2 == 0:
                        nc.scalar.activation(hx[:], ph[:], AF.Copy)
                    else:
                        nc.vector.tensor_copy(hx[:], ph[:])
                    nc.sync.dma_start(out=tbl1[gb * P:(gb + 1) * P, :],
                                      in_=hx[:, 0:TROW1])
                    nc.scalar.dma_start(out=ad1t[gb * P:(gb + 1) * P, :],
                                     in_=hx[:, TROW1:272])

        # ---------------- E1: layer-1 edge phase
        with tc.tile_pool(name="gp", bufs=4) as gp, \
             tc.tile_pool(name="ohp", bufs=4) as ohp, \
             tc.tile_pool(name="sp", bufs=10) as sp, \
             tc.tile_pool(name="fp", bufs=3) as fp, \
             tc.tile_pool(name="oht", bufs=2) as ohtp, \
             tc.tile_pool(name="adb", bufs=4) as adbp, \
             tc.tile_pool(name="pb", bufs=3, space="PSUM") as pbp, \
             tc.tile_pool(name="pt", bufs=1, space="PSUM") as ptp, \
             tc.tile_pool(name="pa", bufs=1, space="PSUM") as pap, \
             tc.tile_pool(name="psb", bufs=1, space="PSUM") as psbp:
            for b in range(NB if phases >= 2 else 0):
                pblk = pbp.tile([P, TROW1], f32, tag="pblk")
                g1 = gp.tile([P, cpc, TROW1], bf16, tag="g")
                adb = adbp.tile([P, 8], bf16, tag="adb")
                nc.gpsimd.indirect_dma_start(
                    out=adb[:], out_offset=None, in_=ad1t[:],
                    in_offset=bass.IndirectOffsetOnAxis(
                        ap=adbi_sb[:, b:b + 1], axis=0))
                for ch in range(cpc):
                    t = b * cpc + ch
                    nc.gpsimd.indirect_dma_start(
                        out=g1[:, ch, :], out_offset=None, in_=tbl1[:],
                        in_offset=bass.IndirectOffsetOnAxis(
                            ap=six_sb[:, t:t + 1], axis=0))
                oh = ohp.tile([P, cpc, P], bf16, tag="oh")
                nc.vector.tensor_tensor(
                    out=oh[:],
                    in0=ld_sb[:, b * cpc:(b + 1) * cpc].unsqueeze(-1)
                        .to_broadcast([P, cpc, P]),
                    in1=iota_sb[:].unsqueeze(1).to_broadcast([P, cpc, P]),
                    op=AL.is_equal)
                pa_blk = pap.tile([P, cpc * 8], f32, tag="pa")
                pstb = psbp.tile([P, cpc, P], bf16, tag="pstb")
                for ch in range(cpc):
                    nc.tensor.transpose(out=pstb[:, ch, :], in_=oh[:, ch, :],
                                        identity=ident_sb[:])
                ohtb = ohtp.tile([P, cpc, P], bf16, tag="ohtb")
                nc.scalar.activation(ohtb[:], pstb[:], AF.Copy)
                for ch in range(cpc):
                    nc.tensor.matmul(out=pa_blk[:, ch * 8:(ch + 1) * 8],
                                     lhsT=ohtb[:, ch, :], rhs=adb[:],
                                     start=True, stop=True)
                hsp = (cpc + 1) // 2
                for lo, hi in ((0, hsp), (hsp, cpc)):
                    w = hi - lo
                    pa_v = pa_blk[:, lo * 8:hi * 8]
                    esc = sp.tile([P, w, 8], f32, tag="esc")
                    nc.vector.tensor_tensor(
                        out=esc[:], in0=g1[:, lo:hi, 256:264],
                        in1=pa_v.rearrange("p (t h) -> p t h", t=w),
                        op=AL.add)
                    es2 = sp.tile([P, w, 8], f32, tag="es2")
                    nc.vector.tensor_scalar_mul(es2[:], esc[:], 0.2)
                    es3 = sp.tile([P, w, 8], f32, tag="es3")
                    nc.vector.tensor_tensor(out=es3[:], in0=esc[:], in1=es2[:],
                                            op=AL.max)
                    nc.scalar.activation(g1[:, lo:hi, 256:264], es3[:], AF.Exp,
                                         bias=msh_sb[:])
                    nc.vector.tensor_tensor(
                        out=g1[:, lo:hi, 0:256]
                            .rearrange("p t (h c) -> p t h c", h=8),
                        in0=g1[:, lo:hi, 0:256]
                            .rearrange("p t (h c) -> p t h c", h=8),
                        in1=g1[:, lo:hi, 256:264].unsqueeze(-1)
                            .to_broadcast([P, w, 8, 32]),
                        op=AL.mult)
                for ch in range(cpc):
                    nc.tensor.matmul(out=pblk[:], lhsT=oh[:, ch, :],
                                     rhs=g1[:, ch, :],
                                     start=(ch == 0), stop=(ch == cpc - 1))
                # park accumulator: reciprocal of denominators + raw sums
                den = sp.tile([P, 8], f32, tag="den")
                nc.vector.tensor_scalar_add(den[:], pblk[:, 256:264], 1e-16)
                nc.vector.reciprocal(rcp1_sb[:, b * 8:(b + 1) * 8], den[:])
                nc.scalar.activation(raw1_sb[:, b * 256:(b + 1) * 256],
                                     pblk[:, 0:256], AF.Copy)

            # pass B: alpha normalize + bias + elu -> h1 (1-block skew so the
            # DVE never waits on the scalar exp of the same block)
            def pb_front(b):
                xb = fp.tile([P, 256], f32, tag="xb")
                nc.vector.tensor_tensor(
                    out=xb[:].rearrange("p (h c) -> p h c", h=8),
                    in0=raw1_sb[:, b * 256:(b + 1) * 256]
                        .rearrange("p (h c) -> p h c", h=8),
                    in1=rcp1_sb[:, b * 8:(b + 1) * 8].unsqueeze(-1)
                        .to_broadcast([P, 8, 32]),
                    op=AL.mult)
                nc.vector.tensor_tensor(out=xb[:], in0=xb[:], in1=b1_sb[:],
                                        op=AL.add)
                m0 = fp.tile([P, 256], f32, tag="m0")
                nc.vector.tensor_scalar(out=m0[:], in0=xb[:], scalar1=0.0,
                                        scalar2=None, op0=AL.min)
                ex = fp.tile([P, 256], f32, tag="ex")
                nc.scalar.activation(ex[:], m0[:], AF.Exp)
                nc.vector.tensor_scalar(out=m0[:], in0=xb[:], scalar1=0.0,
                                        scalar2=None, op0=AL.max)
                return b, m0, ex

            def pb_back(st):
                b, m0, ex = st
                nc.vector.tensor_scalar_add(ex[:], ex[:], -1.0)
                h1b = h1_sb[:, b * 256:(b + 1) * 256]
                nc.vector.tensor_tensor(out=h1b, in0=m0[:], in1=ex[:],
                                        op=AL.add)

            prevb = None
            for b in range(NB if phases >= 2 else 0):
                stb = pb_front(b)
                if prevb is not None:
                    pb_back(prevb)
                prevb = stb
            if prevb is not None:
                pb_back(prevb)

            # h1 transposes fused with the layer-2 node-table matmuls
            for b in range(NB if phases >= 2 else 0):
                for half, dstt in ((0, h1t0), (1, h1t1)):
                    pst2 = ptp.tile([P, P], bf16, tag="pst")
                    nc.tensor.transpose(
                        out=pst2[:],
                        in_=h1_sb[:, b * 256 + half * P: b * 256 + (half + 1) * P],
                        identity=ident_sb[:])
                    nc.scalar.activation(dstt[:, b * P:(b + 1) * P], pst2[:],
                                         AF.Copy)
                if phases >= 3:
                    p2 = ptp.tile([P, TROW2], f32, tag="pst")
                    nc.tensor.matmul(out=p2[:], lhsT=h1t0[:, b * P:(b + 1) * P],
                                     rhs=w2_sb[:, 0, :], start=True, stop=False)
                    nc.tensor.matmul(out=p2[:], lhsT=h1t1[:, b * P:(b + 1) * P],
                                     rhs=w2_sb[:, 1, :], start=False, stop=True)
                    h2x = sp.tile([P, TROW2], bf16, tag="h2x")
                    nc.scalar.activation(h2x[:], p2[:], AF.Copy)
                    nc.sync.dma_start(out=t2shard[b * P:(b + 1) * P, :],
                                      in_=h2x[:])

            # ---------------- AG2
            if phases >= 4:
                nc.gpsimd.collective_compute(
                    "AllGather", AL.bypass, replica_groups=groups,
                    ins=[t2shard[:]], outs=[t2full[:]])

            # ---------------- E2: layer-2 edge phase
            for b in range(NB if phases >= 5 else 0):
                pblk2 = pbp.tile([P, 41], f32, tag="pblk")
                g2 = gp.tile([P, cpc, TROW2], bf16, tag="g2")
                adb2 = adbp.tile([P, TROW2], bf16, tag="adb2")
                nc.sync.dma_start(out=adb2[:],
                                  in_=t2shard[b * P:(b + 1) * P, :])
                for ch in range(cpc):
                    t = b * cpc + ch
                    nc.gpsimd.indirect_dma_start(
                        out=g2[:, ch, :], out_offset=None, in_=t2full[:],
                        in_offset=bass.IndirectOffsetOnAxis(
                            ap=six_sb[:, t:t + 1], axis=0))
                oh = ohp.tile([P, cpc, P], bf16, tag="oh")
                nc.vector.tensor_tensor(
                    out=oh[:],
                    in0=ld_sb[:, b * cpc:(b + 1) * cpc].unsqueeze(-1)
                        .to_broadcast([P, cpc, P]),
                    in1=iota_sb[:].unsqueeze(1).to_broadcast([P, cpc, P]),
                    op=AL.is_equal)
                pa2 = pap.tile([P, cpc], f32, tag="pa")
                pstb = psbp.tile([P, cpc, P], bf16, tag="pstb")
                for ch in range(cpc):
                    nc.tensor.transpose(out=pstb[:, ch, :], in_=oh[:, ch, :],
                                        identity=ident_sb[:])
                ohtb = ohtp.tile([P, cpc, P], bf16, tag="ohtb")
                nc.scalar.activation(ohtb[:], pstb[:], AF.Copy)
                for ch in range(cpc):
                    nc.tensor.matmul(out=pa2[:, ch:ch + 1],
                                     lhsT=ohtb[:, ch, :], rhs=adb2[:, 41:42],
                                     start=True, stop=True)
                hsp = (cpc + 1) // 2
                for lo, hi in ((0, hsp), (hsp, cpc)):
                    w = hi - lo
                    esc = sp.tile([P, w], f32, tag="esc1")
                    nc.vector.tensor_tensor(
                        out=esc[:],
                        in0=g2[:, lo:hi, 40:41].rearrange("p t o -> p (t o)"),
                        in1=pa2[:, lo:hi], op=AL.add)
                    es2 = sp.tile([P, w], f32, tag="es21")
                    nc.vector.tensor_scalar_mul(es2[:], esc[:], 0.2)
                    es3 = sp.tile([P, w], f32, tag="es31")
                    nc.vector.tensor_tensor(out=es3[:], in0=esc[:], in1=es2[:],
                                            op=AL.max)
                    nc.scalar.activation(
                        g2[:, lo:hi, 40:41].rearrange("p t o -> p (t o)"),
                        es3[:], AF.Exp, bias=msh_sb[:])
                    nc.vector.tensor_tensor(
                        out=g2[:, lo:hi, 0:40], in0=g2[:, lo:hi, 0:40],
                        in1=g2[:, lo:hi, 40:41].to_broadcast([P, w, 40]),
                        op=AL.mult)
                for ch in range(cpc):
                    nc.tensor.matmul(out=pblk2[:], lhsT=oh[:, ch, :],
                                     rhs=g2[:, ch, 0:41],
                                     start=(ch == 0), stop=(ch == cpc - 1))
                den = sp.tile([P, 1], f32, tag="den1")
                nc.vector.tensor_scalar_add(den[:], pblk2[:, 40:41], 1e-16)
                nc.vector.reciprocal(rcp2_sb[:, b:b + 1], den[:])
                nc.scalar.activation(raw2_sb[:, b * NCLASS:(b + 1) * NCLASS],
                                     pblk2[:, 0:40], AF.Copy)

            # E2 pass B: normalize + bias + row-max + exp-sum
            for b in range(NB if phases >= 5 else 0):
                o2 = o2_sb[:, b * NCLASS:(b + 1) * NCLASS]
                nc.vector.tensor_tensor(
                    out=o2, in0=raw2_sb[:, b * NCLASS:(b + 1) * NCLASS],
                    in1=rcp2_sb[:, b:b + 1].to_broadcast([P, 40]),
                    op=AL.mult)
                nc.vector.tensor_tensor(out=o2, in0=o2, in1=b2_sb[:], op=AL.add)
                nc.vector.tensor_reduce(out=mmax_sb[:, b:b + 1], in_=o2,
                                        axis=mybir.AxisListType.X, op=AL.max)
                negm = sp.tile([P, 1], f32, tag="negm")
                nc.vector.tensor_scalar_mul(negm[:], mmax_sb[:, b:b + 1], -1.0)
                et = sp.tile([P, NCLASS], f32, tag="et")
                nc.scalar.activation(et[:], o2, AF.Exp, bias=negm[:],
                                     accum_out=ssum_sb[:, b:b + 1])

            if phases >= 5:
                lnS = perm.tile([P, NB], f32)
                nc.scalar.activation(lnS[:], ssum_sb[:], AF.Ln)
                q = perm.tile([P, NB], f32)
                nc.vector.tensor_tensor(out=q[:], in0=mmax_sb[:], in1=lnS[:],
                                        op=AL.add)
                for b in range(NB):
                    o2 = o2_sb[:, b * NCLASS:(b + 1) * NCLASS]
                    lsb = sp.tile([P, NCLASS], f32, tag="lsb")
                    nc.vector.tensor_tensor(out=lsb[:], in0=o2,
                                            in1=q[:, b:b + 1].to_broadcast([P, 40]),
                                            op=AL.subtract)
                    nc.sync.dma_start(out=fin[b * P:(b + 1) * P, :], in_=o2)
                    nc.scalar.dma_start(out=lsm[b * P:(b + 1) * P, :], in_=lsb[:])
    return nc


# ---------------------------------------------------------------------------
def _host_prep(x, edge_index, W1, att_src1, att_dst1, b1, W2, att_src2,
               att_dst2, b2):
    import ml_dtypes
    bf = ml_dtypes.bfloat16

    x = np.asarray(x, dtype=np.float32)
    ei = np.asarray(edge_index)
    W1 = np.asarray(W1, np.float32)
    W2 = np.asarray(W2, np.float32)
    a_s1 = np.asarray(att_src1, np.float32)
    a_d1 = np.asarray(att_dst1, np.float32)
    a_s2 = np.asarray(att_src2, np.float32).reshape(-1)
    a_d2 = np.asarray(att_dst2, np.float32).reshape(-1)
    b1 = np.asarray(b1, np.float32)
    b2 = np.asarray(b2, np.float32)

    loops = np.arange(N, dtype=ei.dtype)
    src = np.concatenate([ei[0], loops])
    dst = np.concatenate([ei[1], loops])
    order = np.argsort(dst, kind="stable")
    src_s = src[order].astype(np.int64)
    dst_s = dst[order].astype(np.int64)

    core = dst_s // SH
    local = dst_s - core * SH
    block = local // P
    slot = local % P
    cb = core * NB + block
    counts = np.bincount(cb, minlength=NCORES * NB)
    cpc = int(np.ceil(counts.max() / P))
    TOT = NB * cpc

    offs = np.zeros(NCORES * NB + 1, np.int64)
    offs[1:] = np.cumsum(counts)
    pos = np.arange(src_s.size, dtype=np.int64) - offs[cb]
    chunk = pos // P
    lane = pos % P
    col = block * cpc + chunk

    ld_all = np.full((NCORES, P, TOT), -1.0, np.float32)
    six_all = np.zeros((NCORES, P, TOT), np.int32)
    dix_all = np.zeros((NCORES, P, TOT), np.int32)
    dixl_all = np.zeros((NCORES, P, TOT), np.int32)
    smap = ((src_s // SH) * SHP + (src_s % SH)).astype(np.int32)
    dmap = (core * SHP + local).astype(np.int32)
    ld_all[core, lane, col] = slot.astype(np.float32)
    six_all[core, lane, col] = smap
    dix_all[core, lane, col] = dmap
    dixl_all[core, lane, col] = (local).astype(np.int32)

    # weights folded with attention vectors
    wa_s1 = np.zeros((NFEAT, 8), np.float32)
    wa_d1 = np.zeros((NFEAT, 8), np.float32)
    for h in range(NHEAD):
        wa_s1[:, h] = W1[:, h * HID:(h + 1) * HID] @ a_s1[h]
        wa_d1[:, h] = W1[:, h * HID:(h + 1) * HID] @ a_d1[h]
    wcat1 = np.concatenate([W1, wa_s1, wa_d1], axis=1).astype(bf)   # [256,272]
    w2cat = np.concatenate([W2, (W2 @ a_s2)[:, None],
                            (W2 @ a_d2)[:, None]], axis=1).astype(bf)

    # replicated x in remapped (core-padded) row order, transposed
    xs_all = np.zeros((ROWS, NFEAT), np.float32)
    for c in range(NCORES):
        xs_all[c * SHP:c * SHP + SH] = x[c * SH:(c + 1) * SH]
    xT = np.ascontiguousarray(xs_all.T).astype(bf)

    iota_row = np.broadcast_to(np.arange(P, dtype=np.float32),
                               (P, P)).astype(bf)
    ident = np.eye(P, dtype=np.float32).astype(bf)
    b1rep = np.broadcast_to(b1, (P, 256)).copy()
    b2rep = np.broadcast_to(b2, (P, NCLASS)).copy()

    pp = np.arange(P, dtype=np.int32)
    in_maps = []
    for c in range(NCORES):
        adblkidx = (c * SHP + (np.arange(NB, dtype=np.int32) * P)[None, :]
                    + pp[:, None])
        in_maps.append({
            "adblkidx": np.ascontiguousarray(adblkidx),
            "xT": xT,
            "wcat1": wcat1, "w2cat": w2cat,
            "b1rep": b1rep, "b2rep": b2rep,
            "iota_row": np.ascontiguousarray(iota_row), "ident": ident,
            "ld": np.ascontiguousarray(ld_all[c]).astype(bf),
            "srcidx": np.ascontiguousarray(six_all[c]),
            "dstidxg": np.ascontiguousarray(dix_all[c]),
            "dstidxl": np.ascontiguousarray(dixl_all[c]),
        })
    return cpc, in_maps


def kernel(**inputs):
    _install_bir_patch()
    from concourse.bass_utils import run_bass_kernel_spmd

    cpc, in_maps = _host_prep(
        inputs["x"], inputs["edge_index"], inputs["W1"], inputs["att_src1"],
        inputs["att_dst1"], inputs["b1"], inputs["W2"], inputs["att_src2"],
        inputs["att_dst2"], inputs["b2"])

    if cpc not in _prog_cache:
        _prog_cache[cpc] = _build_program(cpc)
    nc = _prog_cache[cpc]

    res = run_bass_kernel_spmd(nc, in_maps, list(range(NCORES)))
    fin = np.concatenate([res.results[c]["fin"][:SH] for c in range(NCORES)])
    lsm = np.concatenate([res.results[c]["lsm"][:SH] for c in range(NCORES)])
    return fin, lsm
